# revision 33
# baseline (speedup 1.0000x reference)
"""HardMemory retrieval-KNN kernel for 8 Trainium2 NeuronCores.

Data-parallel: 32 batches sharded 4-per-core; memory bank [1024,512]
replicated.  Each batch (x_b = [C=512, N=4096]) is processed in four
1024-pixel blocks, software-pipelined 4 deep:

  round r:  PE   : gather(r-2) 32mm | sim(r+1) 32mm   (one solid burst)
            Act  : sim psum->bf16 drains (r+1)
            DVE  : max tree + thr fold (r) | out drains (r-2) | compare (r-1)
            Pool : partition_all_reduce max (r)
            DMA  : x/thr prefetch (r+2), output (r-2)

  simT[m,n]  = <x_n, mem_m/||mem_m||>   fp8 DoubleRow matmuls, f32 psum
  cm[n]      = colmax over 8 m-chunks   DVE max tree (bf16 2x)
  cm[0,:]   |= max(cm[0,:], thr')       thr' = nextup(bf16(0.8*||x||)),
                                        host-precomputed, one sbuf row
  cmB[m,n]   = allreduce-max partitions (gpsimd) -> full [128,N] operand
  oh[m,n]    = (sTb == cmB)             exact bf16 compare -> fp8 bitcast
  out[:,n]   = memory^T @ oh            fp8 DR matmuls -> bf16 out

Masked pixels (colmax <= thr) end with cmB = thr' which is strictly
above every sim value, so the onehot is all-zero and the output column
is exactly 0 -- same strict-compare semantics as the reference mask.
The bf16 compare domain is exact (max of bf16 values == some bf16
value).  bf16 1.0 = 0x3F80; its high byte read as fp8e4m3 is 1.875, so
the odd bytes of the compare output form an fp8 onehot scaled by 1.875
(memS2 carries the 1/1.875).
"""

import sys

for _p in ("/opt/trn_rl_repo",):
    if _p not in sys.path:
        sys.path.insert(0, _p)

from contextlib import ExitStack

import ml_dtypes
import numpy as np

import concourse.bass as bass
import concourse.tile as tile
from concourse import bacc, bass_isa, mybir
from concourse.bass_utils import run_bass_kernel_spmd

F32 = mybir.dt.float32
BF16 = mybir.dt.bfloat16
FP8 = mybir.dt.float8e4
AF = mybir.ActivationFunctionType
ALU = mybir.AluOpType
DR = mybir.MatmulPerfMode.DoubleRow

B_FULL, C, H, W = 32, 512, 64, 64
N_PIX = H * W
M = 1024
N_CORES = 8
B_LOC = B_FULL // N_CORES

MC = M // 128            # 8 memory chunks
MJ = MC // 2             # 4 DoubleRow memory pairs
CJ = C // 256            # 2 DoubleRow contraction pairs
BLK = 1024               # pixels per block

# engine split for psum->sbuf drains (GPSIMD has no PSUM access)
SD_ENG = ["act", "act", "act", "dve", "dve", "act", "act", "act"]
OD_ENG = ["act"] * 8


def build_kernel(b_loc=B_LOC, n_pix=N_PIX):
    nblk = (b_loc * n_pix) // BLK

    nc = bacc.Bacc("TRN2", target_bir_lowering=False, debug=False,
                   num_devices=N_CORES)
    xs = nc.dram_tensor("xs", [b_loc, C, n_pix], FP8, kind="ExternalInput")
    memS2_d = nc.dram_tensor("memS2", [MJ, 128, C // 128, 2, 128], FP8,
                             kind="ExternalInput")
    memT2_d = nc.dram_tensor("memT2", [CJ, 128, MC, 2, 128], FP8,
                             kind="ExternalInput")
    thr_d = nc.dram_tensor("thr", [max(nblk, 1), BLK], BF16,
                           kind="ExternalInput")
    out = nc.dram_tensor("out", [b_loc, C, n_pix], BF16,
                         kind="ExternalOutput")

    with tile.TileContext(nc) as tc, ExitStack() as ctx:
        const = ctx.enter_context(tc.tile_pool(name="const", bufs=1))
        mstage = ctx.enter_context(tc.tile_pool(name="mstage", bufs=2))
        mtmp = ctx.enter_context(tc.tile_pool(name="mtmp", bufs=2))
        xio = ctx.enter_context(tc.tile_pool(name="xio", bufs=3))
        stb = ctx.enter_context(tc.tile_pool(name="stb", bufs=3))
        ohb = ctx.enter_context(tc.tile_pool(name="ohb", bufs=2))
        fnd = ctx.enter_context(tc.tile_pool(name="fnd", bufs=2))
        obp = ctx.enter_context(tc.tile_pool(name="obp", bufs=2))
        stats = ctx.enter_context(tc.tile_pool(name="stats", bufs=2))
        # psum (8 banks): sim ring 3x[128,2,512]f32 (6) + gather ring
        # 2x[128,512]f32 (2).  Preproc transposes ride the gather ring.
        psum = ctx.enter_context(
            tc.tile_pool(name="psum", bufs=1, space=bass.MemorySpace.PSUM))

        def drain(which, dst, src):
            if which == "act":
                nc.scalar.activation(dst, src, AF.Copy)
            else:
                nc.vector.tensor_copy(dst, src)

        # ---- memory stationaries, pre-transposed/quantized on host ----
        # Dual-fp8 ldweights needs each [2, 128] stationary block contiguous.
        # memS2[mj][p, ci, i, c] = mem[(2mj+i)*128+p, ci*128+c]/1.875 (gather)
        # memT2[cj][p, mt, i, m] = mem_norm[mt*128+m, (2cj+i)*128+p]  (sim)
        memS2 = [const.tile([128, C // 128, 2, 128], FP8, tag=f"memS2_{mj}",
                            name=f"memS2_{mj}") for mj in range(MJ)]
        memT2 = [const.tile([128, MC, 2, 128], FP8, tag=f"memT2_{cj}",
                            name=f"memT2_{cj}") for cj in range(CJ)]
        nc.sync.dma_start(memT2[0][:, 0:4], memT2_d[0, :, 0:4])
        nc.sync.dma_start(memT2[0][:, 4:8], memT2_d[0, :, 4:8])

        def load_memT2_cj1():
            nc.sync.dma_start(memT2[1][:], memT2_d[1])

        def load_memS2():
            for mj in range(MJ):
                nc.sync.dma_start(memS2[mj][:], memS2_d[mj])

        def blk_addr(k):
            b = k // (n_pix // BLK)
            n0 = (k % (n_pix // BLK)) * BLK
            return b, n0

        def phase_dma(k):
            """Prefetch one block's input (two rounds ahead)."""
            b, n0 = blk_addr(k)
            x4 = xio.tile([128, 4, BLK], FP8, tag="x4", name="x4")
            if k == 0:
                for h in range(2):
                    srch = xs[b, :, n0 + h * 512:n0 + (h + 1) * 512]
                    nc.sync.dma_start(
                        x4[:, :, h * 512:(h + 1) * 512],
                        srch.rearrange("(ch p) n -> p ch n", ch=4))
            else:
                src = xs[b, :, n0:n0 + BLK].rearrange(
                    "(ch p) n -> p ch n", ch=4)
                nc.sync.dma_start(x4[:], src)
            thrR = stats.tile([1, BLK], BF16, tag="thrR", bufs=4,
                              name="thrR")
            nc.sync.dma_start(thrR[:], thr_d[k:k + 1, :])
            return {"k": k, "x4": x4, "thrR": thrR}

        def phase_sim(st):
            """Sim matmuls + psum->sbuf bf16 drains (one round ahead)."""
            x4 = st["x4"]
            sTb = stb.tile([128, MC, BLK], BF16, tag="sTb", name="sTb")
            for mt in range(MC):
                ps = psum.tile([128, 2, 512], F32, tag="sim", bufs=3,
                               name="ps")
                for cj in range(CJ):
                    for h in range(2):
                        nc.tensor.matmul(
                            ps[:, h, :], memT2[cj][:, mt, :, :],
                            x4[:, 2 * cj:2 * cj + 2, h * 512:(h + 1) * 512],
                            start=(cj == 0), stop=(cj == CJ - 1),
                            perf_mode=DR)
                drain(SD_ENG[mt], sTb[:, mt, :],
                      ps[:].rearrange("p a n -> p (a n)"))
            st["sTb"] = sTb

        def phase_find(st):
            """Column max tree, threshold fold, cross-partition allreduce."""
            sTb, thrR = st["sTb"], st["thrR"]
            cmp4 = fnd.tile([128, 4, BLK], BF16, tag="cmp4", name="cmp4")
            nc.vector.tensor_tensor(cmp4[:], sTb[:, 0:4, :], sTb[:, 4:8, :],
                                    ALU.max)
            cmx2 = fnd.tile([128, 2, BLK], BF16, tag="cmx2", name="cmx2")
            nc.vector.tensor_tensor(cmx2[:], cmp4[:, 0:2, :], cmp4[:, 2:4, :],
                                    ALU.max)
            cm = fnd.tile([128, BLK], BF16, tag="cm", name="cm")
            nc.vector.tensor_tensor(cm[:], cmx2[:, 0, :], cmx2[:, 1, :],
                                    ALU.max)
            # fold thr' into one partition; the cross-partition max spreads it
            nc.vector.tensor_tensor(cm[0:1, :], cm[0:1, :], thrR[:], ALU.max)
            cmB = fnd.tile([128, BLK], BF16, tag="cmB", name="cmB")
            for h in range(2):
                nc.gpsimd.partition_all_reduce(
                    cmB[:, h * 512:(h + 1) * 512],
                    cm[:, h * 512:(h + 1) * 512], 128, bass_isa.ReduceOp.max)
            st["cmB"] = cmB

        def phase_oh(st):
            """Onehot: exact bf16 compare (DVE 2x, one shot)."""
            sTb, cmB = st["sTb"], st["cmB"]
            oh = ohb.tile([128, MC, BLK], BF16, tag="oh", name="oh")
            for h in range(2):
                sl = slice(h * 512, (h + 1) * 512)
                mxv = cmB[:, sl].unsqueeze(1).broadcast_to([128, 4, 512])
                nc.vector.tensor_tensor(oh[:, 0:4, sl], sTb[:, 0:4, sl],
                                        mxv, ALU.is_equal)
                nc.vector.tensor_tensor(oh[:, 4:8, sl], sTb[:, 4:8, sl],
                                        mxv, ALU.is_equal)
            oh8 = oh[:].bitcast(FP8).rearrange(
                "p mt (n two) -> p mt n two", two=2)
            st["oh8"] = oh8

        def phase_gather(st):
            """out[c, n] = sum_m mem[m, c] * onehot[m, n], drain, DMA."""
            k, oh8 = st["k"], st["oh8"]
            b, n0 = blk_addr(k)
            ob = obp.tile([128, 4, BLK], BF16, tag="ob", name="ob")
            for ci in range(C // 128):
                pBs = [psum.tile([128, 512], F32, tag="gat", bufs=2,
                                 name="pB") for _ in range(2)]
                for mj in range(MJ):
                    for h in range(2):
                        nc.tensor.matmul(
                            pBs[h][:], memS2[mj][:, ci, :, :],
                            oh8[:, 2 * mj:2 * mj + 2,
                                h * 512:(h + 1) * 512, 1],
                            start=(mj == 0), stop=(mj == MJ - 1),
                            perf_mode=DR)
                for h in range(2):
                    drain(OD_ENG[2 * ci + h],
                          ob[:, ci, h * 512:(h + 1) * 512], pBs[h][:])
                if st.get("last") and ci == 1:
                    dst = out[b, 0:256, n0:n0 + BLK].rearrange(
                        "(ci p) n -> p ci n", ci=2)
                    nc.sync.dma_start(dst, ob[:, 0:2, :])
            if st.get("last"):
                dst = out[b, 256:512, n0:n0 + BLK].rearrange(
                    "(ci p) n -> p ci n", ci=2)
                nc.sync.dma_start(dst, ob[:, 2:4, :])
            else:
                dst = out[b, :, n0:n0 + BLK].rearrange(
                    "(ci p) n -> p ci n", ci=4)
                nc.sync.dma_start(dst, ob[:])

        # ---- main loop, pipelined 4 deep ----
        # Emission order per round r gives each in-order engine queue only
        # work whose deps are already met at round start:
        #   PE  : gather(r-2) | sim(r+1)      DVE: tree(r) | od(r-2) | cmp(r-1)
        #   Act : drains(r+1)                 Pool: allreduce(r)
        states = [None] * nblk
        states[0] = phase_dma(0)
        load_memT2_cj1()
        load_memS2()
        if nblk > 1:
            states[1] = phase_dma(1)
        phase_sim(states[0])
        for r in range(nblk):
            if r + 2 < nblk:
                states[r + 2] = phase_dma(r + 2)
            phase_find(states[r])
            if r + 1 < nblk:
                phase_sim(states[r + 1])
            if r >= 1:
                phase_oh(states[r - 1])
            if r >= 2:
                phase_gather(states[r - 2])
                states[r - 2] = None
        phase_oh(states[nblk - 1])
        if nblk >= 2:
            phase_gather(states[nblk - 2])
        states[nblk - 1]["last"] = True
        phase_gather(states[nblk - 1])

    nc.compile()
    return nc


_NC_CACHE = {}


def _get_nc(b_loc=B_LOC, n_pix=N_PIX):
    key = (b_loc, n_pix)
    if key not in _NC_CACHE:
        _NC_CACHE[key] = build_kernel(*key)
    return _NC_CACHE[key]


def make_aux(x_flat_f32, b_loc, n_pix):
    """Host-side aux: thr rows (nextup'd bf16) and the identity."""
    nblk = (b_loc * n_pix) // BLK
    norms = np.sqrt(np.square(x_flat_f32).sum(axis=1))      # [b_loc, n_pix]
    thr = (0.8 * norms).reshape(nblk, BLK).astype(ml_dtypes.bfloat16)
    # strictly-next bf16 so masked columns can never compare equal
    tbits = thr.view(np.uint16) + 1
    thr = tbits.view(ml_dtypes.bfloat16)
    ident = np.eye(128, dtype=ml_dtypes.bfloat16)
    return thr, ident


def run_on_hw(x_flat, memory, b_loc=B_LOC, n_pix=N_PIX, trace=False,
              **spmd_kwargs):
    """x_flat: [N_CORES*b_loc, C, n_pix] f32. Returns (out_full, results)."""
    nc = _get_nc(b_loc, n_pix)
    x_f8 = x_flat.astype(ml_dtypes.float8_e4m3)
    in_maps = []
    for c in range(N_CORES):
        xc = x_flat[c * b_loc:(c + 1) * b_loc]
        thr, ident = make_aux(xc, b_loc, n_pix)
        in_maps.append({
            "xs": np.ascontiguousarray(x_f8[c * b_loc:(c + 1) * b_loc]),
            "memory": memory,
            "identity": ident,
            "thr": thr,
        })
    res = run_bass_kernel_spmd(nc, in_maps, list(range(N_CORES)),
                               trace=trace, **spmd_kwargs)
    outs = [np.asarray(res.results[c]["out"]).astype(np.float32)
            for c in range(N_CORES)]
    return np.concatenate(outs, axis=0), res


def kernel(x, memory):
    x = np.asarray(x, dtype=np.float32)
    memory = np.asarray(memory, dtype=np.float32)
    B, C_, H_, W_ = x.shape
    x_flat = np.ascontiguousarray(x.reshape(B, C_, H_ * W_))
    out_flat, _ = run_on_hw(x_flat, memory)
    return out_flat.reshape(B, C_, H_, W_)
